# revision 8
# baseline (speedup 1.0000x reference)
"""Multi-head attention (B=2, H=16, S=2048, D=64) on 8 trn2 NeuronCores.

Sharding: the 32 (b, h) head-units are split 4-per-core (head/data parallel,
no cross-core comms).  Per core, for each head:

  scoresT[k, q] = sum_d K[k, d] Q[q, d]            (PE, contract=64, row-packed 2x)
  pT[k, q]      = exp(scoresT/8) * keep01T[k, q]   (split across ACT and DVE,
                                                    see below)
  OT'[m, q]     = sum_k V'[k, m] pT[k, q]          (PE, V' = [V | ones] so row 64
                                                    of OT' is the softmax denom Z)
  out[q, d]     = OT'[d, q] / OT'[64, q]           (host-side: O(S*D) divide +
                                                    transpose while unsharding)

The exp+mask stage is the throughput bottleneck if left on one engine
(ACT exp alone measures ~128us/core; DVE mask ~71us).  So the 16 key-chunks
of each (sqb, h) tile are split between two independent paths:

  chunks [0, ACT_CHUNKS):   ACT exp (psum f32 -> fp16, scale fused), then
                            DVE fp16 0/1-mask multiply (2x DVE mode).
  chunks [ACT_CHUNKS, 16):  one full-rate DVE op per psum group:
                            int16 = (psum * A) + B_kq   (scalar_tensor_tensor)
                            with A = 1024*log2(e)/8.  For kept positions
                            B_kq ~ 15316 makes the int16 result the BIT
                            PATTERN of fp16 exp(s/8) (Schraudolph trick,
                            +-3% rel err).  For masked positions
                            B_kq = -42000 drives the fp32 result below
                            -32768; the int16 convert SATURATES, giving
                            0x8000 = fp16 -0.0 (verified on HW).  One DVE
                            pass fuses exp+scale+mask.

The same mk input carries 0/1 fp16 for ACT chunks and B0/BNEG fp16 for DVE
chunks (disjoint chunk ranges, host-prepped).

Working in the transposed-score layout means softmax needs no reductions at
all (Z rides along in the PV matmul) and no S x S transposes anywhere.
"""

import numpy as np

import concourse.bass as bass  # noqa: F401  (engine types resolve through nc)
import concourse.mybir as mybir
import concourse.tile as tile
from concourse import bacc
from concourse.bass_utils import run_bass_kernel_spmd

B, H, S, D = 2, 16, 2048, 64
N_CORES = 8
HPC = (B * H) // N_CORES  # heads per core

SQ = 512        # query-block width (one fp32 PSUM bank)
CK = 128        # key-chunk height (PSUM partition dim)
# Key chunks per exp group: 3-bank [128, 1536] PSUM groups maximize the ACT
# call size (per-ACTIVATE overhead is ~0.3us on HW) within the 8-bank budget
# (2x 3-bank qk slots + 2x 1-bank PV accumulators).
GROUPS = [(0, 3), (3, 3), (6, 3), (9, 3), (12, 3), (15, 1)]
HALVES = [(0, 0, 2), (1, 2, 6)]   # (half idx, first group, end group)
VW = D + 2      # V' width: 64 V columns + ones column + pad (66)

ACT_CHUNKS = 12  # chunks [0, a) on the ACT path, [a, 16) on the DVE trick

# DVE exp bit-trick constants (fp16 bit pattern = A*s + B0 for kept, BNEG
# saturates the int16 convert to 0x8000 = -0.0 for masked).
TRICK_A = float(1024 * np.log2(np.e) / 8.0)
TRICK_B0 = 15360.0 - 44.11
TRICK_BNEG = -42000.0

f32 = mybir.dt.float32
f16 = mybir.dt.float16
i16 = mybir.dt.int16
FT = mybir.ActivationFunctionType


def build_nc(hpc=HPC, s=S, loop_n=None, ablate=(), loop_stagger=False,
             act_chunks=ACT_CHUNKS):
    """Build the per-core Bass program (identical on all 8 cores).

    loop_n: if set, wrap the whole body in an on-device For_i loop that
    recomputes the same output loop_n times — a perf-measurement rig that
    lets wall-clock deltas between two loop_n values cancel host/RPC
    overheads (this container has no NTFF profile path).

    ablate: perf-debug only — subset of {"qk", "act", "mask", "pv", "tail"}
    to skip emitting, isolating per-engine throughput on HW. Output is
    garbage when non-empty.
    """
    nsq = s // SQ
    nck = s // CK
    groups = [(c0, n) for c0, n in GROUPS if c0 + n <= nck] if nck == 16 else [
        (c, 1) for c in range(nck)]
    if nck != 16:
        act_chunks = nck  # small-s debug builds: ACT path only
    ablate = set(ablate)

    nc = bacc.Bacc("TRN2", target_bir_lowering=False, debug=False)

    qt_d = nc.dram_tensor("qt", [hpc, D, s], f16, kind="ExternalInput")
    kt_d = nc.dram_tensor("kt", [hpc, D, s], f16, kind="ExternalInput")
    vp_d = nc.dram_tensor("vp", [hpc, CK, nck * VW], f16, kind="ExternalInput")
    mk_d = nc.dram_tensor("mk", [nsq, CK, nck * SQ], f16, kind="ExternalInput")
    o_d = nc.dram_tensor("o", [hpc, nsq, VW, SQ], f32, kind="ExternalOutput")

    with tile.TileContext(nc) as tc:
        if ablate:
            tc.race_detector_enabled = False
        with (
            tc.tile_pool(name="heads", bufs=hpc) as head_pool,
            tc.tile_pool(name="mask", bufs=nsq) as mask_pool,
            tc.tile_pool(name="pt", bufs=2) as pt_pool,
            tc.tile_pool(name="tail", bufs=2) as tail_pool,
            tc.tile_pool(name="qk_ps", bufs=2, space="PSUM") as qk_pool,
            tc.tile_pool(name="o_ps", bufs=2, space="PSUM") as o_pool,
        ):
            qt_t, kt_t, vp_t = [], [], []
            for h in range(hpc):
                q_t = head_pool.tile([128, s], f16, name=f"qt_sb{h}", tag="qt")
                k_t = head_pool.tile([128, s], f16, name=f"kt_sb{h}", tag="kt")
                v_t = head_pool.tile([CK, nck * VW], f16, name=f"vp_sb{h}", tag="vp")
                # Q^T/K^T live duplicated in both partition halves so the two
                # row-packed K=64 matmuls can run concurrently on the PE.
                nc.sync.dma_start(out=q_t[0:D, :], in_=qt_d[h, :, :])
                nc.sync.dma_start(out=q_t[D:128, :], in_=qt_d[h, :, :])
                nc.sync.dma_start(out=k_t[0:D, :], in_=kt_d[h, :, :])
                nc.sync.dma_start(out=k_t[D:128, :], in_=kt_d[h, :, :])
                nc.sync.dma_start(out=v_t[:, :], in_=vp_d[h, :, :])
                qt_t.append(q_t)
                kt_t.append(k_t)
                vp_t.append(v_t)

            # The whole mask/bias image fits in SBUF — load it once, outside
            # any measurement loop (saves 8MB of DMA per pass).
            mk_t = {}     # sqb -> mask tile [128, nck*SQ] (chunk-major columns)
            for sqb in range(nsq):
                mk = mask_pool.tile([CK, nck * SQ], f16, name=f"mk_sb{sqb}",
                                    tag="mk")
                nc.sync.dma_start(out=mk[:, :], in_=mk_d[sqb, :, :])
                mk_t[sqb] = mk

            pt_t = {}     # (sqb, h) -> p^T tile [128, nck*SQ] fp16
            o_ps = {}     # (sqb, h) -> PSUM accumulator [VW, SQ]

            def emit_qk_group(sqb, h, c0, n):
                """QK matmuls + exp (ACT or DVE-trick) for chunks [c0, c0+n)."""
                qk = None
                if "qk" not in ablate:
                    qk = qk_pool.tile([128, n * SQ], f32,
                                      name=f"qk_{sqb}_{h}_{c0}", tag="qk",
                                      padded_shape=[128, 3 * SQ])
                for j in range(n):
                    if "qk" in ablate:
                        break
                    c = c0 + j
                    bp = 64 * (j % 2)  # row-group for PE packing
                    nc.tensor.matmul(
                        qk[:, j * SQ:(j + 1) * SQ],
                        lhsT=kt_t[h][bp:bp + D, c * CK:(c + 1) * CK],
                        rhs=qt_t[h][bp:bp + D, sqb * SQ:(sqb + 1) * SQ],
                        start=True,
                        stop=True,
                        tile_position=(bp, 0),
                    )
                pt = pt_t[(sqb, h)]
                if pt is None:
                    return
                # ACT-path sub-range [c0, min(a, c0+n)): spline exp.
                a_hi = min(act_chunks, c0 + n)
                if a_hi > c0 and "act" not in ablate:
                    lo, hi = c0 * SQ, a_hi * SQ
                    qlo, qhi = 0, (a_hi - c0) * SQ
                    act_in = qk[:, qlo:qhi] if qk is not None else mk_t[sqb][:, lo:hi]
                    nc.scalar.activation(pt[:, lo:hi], act_in, FT.Exp, scale=0.125)
                # DVE-path sub-range [max(a, c0), c0+n): fused bit-trick.
                d_lo = max(act_chunks, c0)
                if d_lo < c0 + n and "trick" not in ablate:
                    lo, hi = d_lo * SQ, (c0 + n) * SQ
                    qlo, qhi = (d_lo - c0) * SQ, n * SQ
                    trick_in = (qk[:, qlo:qhi] if qk is not None
                                else mk_t[sqb][:, lo:hi])
                    nc.vector.scalar_tensor_tensor(
                        pt[:, lo:hi].bitcast(i16), trick_in, TRICK_A,
                        mk_t[sqb][:, lo:hi],
                        op0=mybir.AluOpType.mult, op1=mybir.AluOpType.add,
                    )

            def emit_mask(sqb, h, clo, chi):
                """Apply the 0/1 keep-mask to ACT-path chunk cols [clo, chi)
                of p^T in one DVE pass (per-call overhead is ~0.4us; batch
                big). DVE-trick chunks already carry the mask."""
                if "mask" in ablate:
                    return
                chi = min(chi, act_chunks)
                if chi <= clo:
                    return
                pt = pt_t[(sqb, h)]
                lo, hi = clo * SQ, chi * SQ
                nc.vector.tensor_tensor(
                    pt[:, lo:hi], pt[:, lo:hi], mk_t[sqb][:, lo:hi],
                    op=mybir.AluOpType.mult,
                )

            def emit_pv(sqb, h, clo, chi):
                """PV matmuls for chunks [clo, chi), accumulating."""
                if "pv" in ablate:
                    return
                pt = pt_t[(sqb, h)]
                if {"act", "mask", "trick"} <= ablate:
                    pt = mk_t[sqb]  # stand-in written tile for PE-only ablations
                ops = o_ps[(sqb, h)]
                for c in range(clo, chi):
                    nc.tensor.matmul(
                        ops[:, :],
                        lhsT=vp_t[h][:, c * VW:c * VW + VW],
                        rhs=pt[:, c * SQ:(c + 1) * SQ],
                        start=(c == 0),
                        stop=(c == nck - 1),
                    )

            def emit_tail(sqb, h):
                """Evacuate O^T' (unnormalized + Z row) and store."""
                if "tail" in ablate:
                    return
                ops = (o_ps[(sqb, h)][:, :] if "pv" not in ablate
                       else mk_t[sqb][0:VW, 0:SQ])  # stand-in for DVE timing
                ot = tail_pool.tile([VW, SQ], f32, name=f"ot_{sqb}_{h}", tag="ot")
                nc.vector.tensor_copy(ot[:, :], ops)
                nc.sync.dma_start(out=o_d[h, sqb, :, :], in_=ot[:, :])

            # Group-granular software pipeline over (sqb, h) units: unit u's
            # QK/exp/trick groups interleave with unit u-1's PV groups on the
            # PE stream.  This keeps the next unit's scores flowing to ACT
            # early (no ACT bubble while PE does a monolithic PV block) and
            # fills PE slot-wait gaps with PV work whose deps (pt of u-1)
            # resolved a full unit ago — no in-order head-of-line stalls.
            ng = len(groups)
            # mask emission points: after the last ACT group of each half
            # (chunks 0-5 after group 1, chunks 6..a after the last group
            # containing ACT chunks).
            # mask(h1) is deliberately emitted after the trick groups: the
            # tricks release the PSUM slots the next unit's first QK groups
            # need, so they must not queue behind a 1.7us mask on the
            # in-order DVE.
            mask_at = {}
            if nck == 16:
                mask_at[1] = (0, 6)
                mask_at[ng - 1] = (6, 16)
            else:
                mask_at[ng - 1] = (0, nck)

            def alloc_unit(sqb, h):
                if not ({"act", "mask", "trick"} <= ablate):
                    pt_t[(sqb, h)] = pt_pool.tile(
                        [128, nck * SQ], f16, name=f"pt_{sqb}_{h}", tag="pt")
                else:
                    pt_t[(sqb, h)] = None
                if "pv" not in ablate:
                    o_ps[(sqb, h)] = o_pool.tile(
                        [VW, SQ], f32, name=f"ops_{sqb}_{h}", tag="ops")

            def emit_all():
                units = [(sqb, h)
                         for sqb in range(nsq) for h in range(hpc)]
                for u, (sqb, h) in enumerate(units):
                    alloc_unit(sqb, h)
                    for gi, (c0, n) in enumerate(groups):
                        emit_qk_group(sqb, h, c0, n)
                        if gi in mask_at:
                            emit_mask(sqb, h, *mask_at[gi])
                        if u >= 1:
                            pq, ph = units[u - 1]
                            emit_pv(pq, ph, c0, c0 + n)
                            if gi == ng - 1:
                                emit_tail(pq, ph)
                for gi, (c0, n) in enumerate(groups):
                    pq, ph = units[-1]
                    emit_pv(pq, ph, c0, c0 + n)
                    if gi == ng - 1:
                        emit_tail(pq, ph)

            if loop_n is None:
                emit_all()
            else:
                hints = (mybir.EngineType.PE, mybir.EngineType.Activation,
                         mybir.EngineType.DVE)
                with tc.For_i(0, loop_n, 1, hint_engines=hints,
                              staggered_reset=bool(loop_stagger)):
                    emit_all()

    nc.finalize()
    return nc


def shard_inputs(K, Q, V, mask, hpc=HPC, s=S, n_cores=N_CORES,
                 act_chunks=ACT_CHUNKS):
    """Full inputs -> per-core in_maps with device-friendly host layouts."""
    nsq = s // SQ
    nck = s // CK
    n_units = n_cores * hpc
    Kf = np.asarray(K, np.float32).reshape(n_units, s, D)
    Qf = np.asarray(Q, np.float32).reshape(n_units, s, D)
    Vf = np.asarray(V, np.float32).reshape(n_units, s, D)
    keepT = (~np.asarray(mask).reshape(s, s)).T  # [k, q], True = attend
    # chunk-major [nck, CK, nsq, SQ] view of keep^T
    keep_cm = keepT.reshape(nck, CK, nsq, SQ)
    mk_vals = np.where(keep_cm, np.float32(1.0), np.float32(0.0))
    if nck == 16 and act_chunks < nck:
        # DVE-trick chunks carry the exp bias instead of 0/1
        mk_vals[act_chunks:] = np.where(
            keep_cm[act_chunks:], np.float32(TRICK_B0), np.float32(TRICK_BNEG))
    mk_host = np.ascontiguousarray(
        mk_vals.astype(np.float16)
        .transpose(2, 1, 0, 3)
        .reshape(nsq, CK, nck * SQ)
    )
    in_maps = []
    for c in range(n_cores):
        sl = slice(c * hpc, (c + 1) * hpc)
        qt = np.ascontiguousarray(Qf[sl].transpose(0, 2, 1)).astype(np.float16)
        kt = np.ascontiguousarray(Kf[sl].transpose(0, 2, 1)).astype(np.float16)
        vp = np.zeros((hpc, s, VW), np.float16)
        vp[:, :, :D] = Vf[sl]
        vp[:, :, D] = 1.0
        vp = np.ascontiguousarray(
            vp.reshape(hpc, nck, CK, VW).transpose(0, 2, 1, 3)
            .reshape(hpc, CK, nck * VW)
        )
        in_maps.append({"qt": qt, "kt": kt, "vp": vp, "mk": mk_host})
    return in_maps


_NC_CACHE = {}


def _get_nc():
    if "nc" not in _NC_CACHE:
        _NC_CACHE["nc"] = build_nc()
    return _NC_CACHE["nc"]


def run_sharded(in_maps, trace=False, **kwargs):
    return run_bass_kernel_spmd(
        _get_nc(), in_maps, core_ids=list(range(N_CORES)), trace=trace, **kwargs
    )


def unshard_output(per_core_raw, hpc=HPC, s=S):
    """[hpc, nsq, VW, SQ] raw blocks per core -> [n*hpc, s, D] normalized.

    Row D of each block is the softmax denominator Z; dividing and
    transposing here is O(S*D) host work (same order as unsharding).
    """
    n = len(per_core_raw)
    out = np.empty((n * hpc, s, D), np.float32)
    for c, o in enumerate(per_core_raw):
        ot = o[:, :, :D, :] / o[:, :, D:D + 1, :]   # [hpc, nsq, D, SQ]
        out[c * hpc:(c + 1) * hpc] = (
            ot.transpose(0, 1, 3, 2).reshape(hpc, s, D))
    return out


def assemble_output(results):
    out = unshard_output([results[c]["o"] for c in range(N_CORES)])
    return out.reshape(B, H, S, D)


def kernel(K, Q, V, mask):
    in_maps = shard_inputs(K, Q, V, mask)
    res = run_sharded(in_maps)
    return assemble_output(res.results)


# revision 10
# speedup vs baseline: 1.0290x; 1.0290x over previous
"""Multi-head attention (B=2, H=16, S=2048, D=64) on 8 trn2 NeuronCores.

Sharding: the 32 (b, h) head-units are split 4-per-core (head/data parallel,
no cross-core comms).  Per core, for each head:

  scoresT[k, q] = sum_d K[k, d] Q[q, d]            (PE, contract=64, row-packed 2x)
  pT[k, q]      = exp(scoresT/8) * keep01T[k, q]   (split across ACT and DVE,
                                                    see below)
  OT'[m, q]     = sum_k V'[k, m] pT[k, q]          (PE, V' = [V | ones] so row 64
                                                    of OT' is the softmax denom Z)
  out[q, d]     = OT'[d, q] / OT'[64, q]           (host-side: O(S*D) divide +
                                                    transpose while unsharding)

The exp+mask stage is the throughput bottleneck if left on one engine
(ACT exp alone measures ~128us/core; DVE mask ~71us).  So the 16 key-chunks
of each (sqb, h) tile are split between two independent paths:

  chunks [0, ACT_CHUNKS):   ACT exp (psum f32 -> fp16, scale fused), then
                            DVE fp16 0/1-mask multiply (2x DVE mode).
  chunks [ACT_CHUNKS, 16):  one full-rate DVE op per psum group:
                            int16 = (psum * A) + B_kq   (scalar_tensor_tensor)
                            with A = 1024*log2(e)/8.  For kept positions
                            B_kq ~ 15316 makes the int16 result the BIT
                            PATTERN of fp16 exp(s/8) (Schraudolph trick,
                            +-3% rel err).  For masked positions
                            B_kq = -42000 drives the fp32 result below
                            -32768; the int16 convert SATURATES, giving
                            0x8000 = fp16 -0.0 (verified on HW).  One DVE
                            pass fuses exp+scale+mask.

The same mk input carries 0/1 fp16 for ACT chunks and B0/BNEG fp16 for DVE
chunks (disjoint chunk ranges, host-prepped).

Working in the transposed-score layout means softmax needs no reductions at
all (Z rides along in the PV matmul) and no S x S transposes anywhere.
"""

import numpy as np

import concourse.bass as bass  # noqa: F401  (engine types resolve through nc)
import concourse.mybir as mybir
import concourse.tile as tile
from concourse import bacc
from concourse.bass_utils import run_bass_kernel_spmd

B, H, S, D = 2, 16, 2048, 64
N_CORES = 8
HPC = (B * H) // N_CORES  # heads per core

SQ = 512        # query-block width (one fp32 PSUM bank)
CK = 128        # key-chunk height (PSUM partition dim)
# Key chunks per exp group: 3-bank [128, 1536] PSUM groups maximize the ACT
# call size (per-ACTIVATE overhead is ~0.3us on HW) within the 8-bank budget
# (2x 3-bank qk slots + 2x 1-bank PV accumulators).
GROUPS = [(0, 3), (3, 3), (6, 3), (9, 3), (12, 1), (13, 1), (14, 1), (15, 1)]
VW = D + 2      # V' width: 64 V columns + ones column + pad (66)

ACT_CHUNKS = 12  # chunks [0, a) on the ACT path, [a, 16) on the DVE trick

# DVE exp bit-trick constants (fp16 bit pattern = A*s + B0 for kept, BNEG
# saturates the int16 convert to 0x8000 = -0.0 for masked).
TRICK_A = float(1024 * np.log2(np.e) / 8.0)
TRICK_B0 = 15360.0 - 44.11
TRICK_BNEG = -42000.0

f32 = mybir.dt.float32
f16 = mybir.dt.float16
i16 = mybir.dt.int16
FT = mybir.ActivationFunctionType


def build_nc(hpc=HPC, s=S, loop_n=None, ablate=(), loop_stagger=False,
             act_chunks=ACT_CHUNKS):
    """Build the per-core Bass program (identical on all 8 cores).

    loop_n: if set, wrap the whole body in an on-device For_i loop that
    recomputes the same output loop_n times — a perf-measurement rig that
    lets wall-clock deltas between two loop_n values cancel host/RPC
    overheads (this container has no NTFF profile path).

    ablate: perf-debug only — subset of {"qk", "act", "mask", "pv", "tail"}
    to skip emitting, isolating per-engine throughput on HW. Output is
    garbage when non-empty.
    """
    nsq = s // SQ
    nck = s // CK
    groups = [(c0, n) for c0, n in GROUPS if c0 + n <= nck] if nck == 16 else [
        (c, 1) for c in range(nck)]
    if nck != 16:
        act_chunks = nck  # small-s debug builds: ACT path only
    ablate = set(ablate)

    nc = bacc.Bacc("TRN2", target_bir_lowering=False, debug=False)

    qt_d = nc.dram_tensor("qt", [hpc, D, s], f16, kind="ExternalInput")
    kt_d = nc.dram_tensor("kt", [hpc, D, s], f16, kind="ExternalInput")
    vp_d = nc.dram_tensor("vp", [hpc, CK, nck * VW], f16, kind="ExternalInput")
    mk_d = nc.dram_tensor("mk", [nsq, CK, nck * SQ], f16, kind="ExternalInput")
    o_d = nc.dram_tensor("o", [hpc, nsq, VW, SQ], f32, kind="ExternalOutput")

    with tile.TileContext(nc) as tc:
        if ablate:
            tc.race_detector_enabled = False
        with (
            tc.tile_pool(name="heads", bufs=hpc) as head_pool,
            tc.tile_pool(name="mask", bufs=nsq) as mask_pool,
            tc.tile_pool(name="pt", bufs=2) as pt_pool,
            tc.tile_pool(name="tail", bufs=2) as tail_pool,
            tc.tile_pool(name="qk_ps", bufs=2, space="PSUM") as qk_pool,
            tc.tile_pool(name="tk_ps", bufs=1, space="PSUM") as tk_pool,
            tc.tile_pool(name="o_ps", bufs=1, space="PSUM") as o_pool,
        ):
            qt_t, kt_t, vp_t = [], [], []
            for h in range(hpc):
                q_t = head_pool.tile([128, s], f16, name=f"qt_sb{h}", tag="qt")
                k_t = head_pool.tile([128, s], f16, name=f"kt_sb{h}", tag="kt")
                v_t = head_pool.tile([CK, nck * VW], f16, name=f"vp_sb{h}", tag="vp")
                # Q^T/K^T live duplicated in both partition halves so the two
                # row-packed K=64 matmuls can run concurrently on the PE.
                nc.sync.dma_start(out=q_t[0:D, :], in_=qt_d[h, :, :])
                nc.sync.dma_start(out=q_t[D:128, :], in_=qt_d[h, :, :])
                nc.sync.dma_start(out=k_t[0:D, :], in_=kt_d[h, :, :])
                nc.sync.dma_start(out=k_t[D:128, :], in_=kt_d[h, :, :])
                nc.sync.dma_start(out=v_t[:, :], in_=vp_d[h, :, :])
                qt_t.append(q_t)
                kt_t.append(k_t)
                vp_t.append(v_t)

            # The whole mask/bias image fits in SBUF — load it once, outside
            # any measurement loop (saves 8MB of DMA per pass).
            mk_t = {}     # sqb -> mask tile [128, nck*SQ] (chunk-major columns)
            for sqb in range(nsq):
                mk = mask_pool.tile([CK, nck * SQ], f16, name=f"mk_sb{sqb}",
                                    tag="mk")
                nc.sync.dma_start(out=mk[:, :], in_=mk_d[sqb, :, :])
                mk_t[sqb] = mk

            pt_t = {}     # (sqb, h) -> p^T tile [128, nck*SQ] fp16
            o_ps = {}     # (sqb, h) -> PSUM accumulator [VW, SQ]

            def emit_qk_group(sqb, h, c0, n):
                """QK matmuls + exp (ACT or DVE-trick) for chunks [c0, c0+n)."""
                qk = None
                is_trick = c0 >= act_chunks and nck == 16
                if "qk" not in ablate:
                    pool = tk_pool if is_trick else qk_pool
                    qk = pool.tile([128, n * SQ], f32,
                                   name=f"qk_{sqb}_{h}_{c0}", tag="qk",
                                   padded_shape=[128, (1 if is_trick else 3) * SQ])
                for j in range(n):
                    if "qk" in ablate:
                        break
                    c = c0 + j
                    bp = 64 * (c % 2)  # row-group for PE packing
                    nc.tensor.matmul(
                        qk[:, j * SQ:(j + 1) * SQ],
                        lhsT=kt_t[h][bp:bp + D, c * CK:(c + 1) * CK],
                        rhs=qt_t[h][bp:bp + D, sqb * SQ:(sqb + 1) * SQ],
                        start=True,
                        stop=True,
                        tile_position=(bp, 0),
                    )
                pt = pt_t[(sqb, h)]
                if pt is None:
                    return
                # ACT-path sub-range [c0, min(a, c0+n)): spline exp.
                a_hi = min(act_chunks, c0 + n)
                if a_hi > c0 and "act" not in ablate:
                    lo, hi = c0 * SQ, a_hi * SQ
                    qlo, qhi = 0, (a_hi - c0) * SQ
                    act_in = qk[:, qlo:qhi] if qk is not None else mk_t[sqb][:, lo:hi]
                    nc.scalar.activation(pt[:, lo:hi], act_in, FT.Exp, scale=0.125)
                # DVE-path sub-range [max(a, c0), c0+n): fused bit-trick.
                d_lo = max(act_chunks, c0)
                if d_lo < c0 + n and "trick" not in ablate:
                    lo, hi = d_lo * SQ, (c0 + n) * SQ
                    qlo, qhi = (d_lo - c0) * SQ, n * SQ
                    trick_in = (qk[:, qlo:qhi] if qk is not None
                                else mk_t[sqb][:, lo:hi])
                    nc.vector.scalar_tensor_tensor(
                        pt[:, lo:hi].bitcast(i16), trick_in, TRICK_A,
                        mk_t[sqb][:, lo:hi],
                        op0=mybir.AluOpType.mult, op1=mybir.AluOpType.add,
                    )

            def emit_mask(sqb, h, clo, chi):
                """Apply the 0/1 keep-mask to ACT-path chunk cols [clo, chi)
                of p^T in one DVE pass (per-call overhead is ~0.4us; batch
                big). DVE-trick chunks already carry the mask."""
                if "mask" in ablate:
                    return
                chi = min(chi, act_chunks)
                if chi <= clo:
                    return
                pt = pt_t[(sqb, h)]
                lo, hi = clo * SQ, chi * SQ
                nc.vector.tensor_tensor(
                    pt[:, lo:hi], pt[:, lo:hi], mk_t[sqb][:, lo:hi],
                    op=mybir.AluOpType.mult,
                )

            def emit_pv(sqb, h, clo, chi):
                """PV matmuls for chunks [clo, chi), accumulating."""
                if "pv" in ablate:
                    return
                pt = pt_t[(sqb, h)]
                if {"act", "mask", "trick"} <= ablate:
                    pt = mk_t[sqb]  # stand-in written tile for PE-only ablations
                ops = o_ps[(sqb, h)]
                for c in range(clo, chi):
                    nc.tensor.matmul(
                        ops[:, :],
                        lhsT=vp_t[h][:, c * VW:c * VW + VW],
                        rhs=pt[:, c * SQ:(c + 1) * SQ],
                        start=(c == 0),
                        stop=(c == nck - 1),
                    )

            def emit_tail(sqb, h):
                """Evacuate O^T' (unnormalized + Z row) and store."""
                if "tail" in ablate:
                    return
                ops = (o_ps[(sqb, h)][:, :] if "pv" not in ablate
                       else mk_t[sqb][0:VW, 0:SQ])  # stand-in for DVE timing
                ot = tail_pool.tile([VW, SQ], f32, name=f"ot_{sqb}_{h}", tag="ot")
                nc.vector.tensor_copy(ot[:, :], ops)
                nc.sync.dma_start(out=o_d[h, sqb, :, :], in_=ot[:, :])

            # Group-granular software pipeline over (sqb, h) units: unit u's
            # QK/exp/trick groups interleave with unit u-1's PV groups on the
            # PE stream.  This keeps the next unit's scores flowing to ACT
            # early (no ACT bubble while PE does a monolithic PV block) and
            # fills PE slot-wait gaps with PV work whose deps (pt of u-1)
            # resolved a full unit ago — no in-order head-of-line stalls.
            ng = len(groups)
            # mask emission points: after the last ACT group of each half
            # (chunks 0-5 after group 1, chunks 6..a after the last group
            # containing ACT chunks).
            # mask(h1) is deliberately emitted after the trick groups: the
            # tricks release the PSUM slots the next unit's first QK groups
            # need, so they must not queue behind a 1.7us mask on the
            # in-order DVE.
            mask_at = {}
            if nck == 16:
                mask_at[1] = (0, 6)
                mask_at[ng - 1] = (6, 16)
            else:
                mask_at[ng - 1] = (0, nck)

            def alloc_unit(sqb, h):
                if not ({"act", "mask", "trick"} <= ablate):
                    pt_t[(sqb, h)] = pt_pool.tile(
                        [128, nck * SQ], f16, name=f"pt_{sqb}_{h}", tag="pt")
                else:
                    pt_t[(sqb, h)] = None
                if "pv" not in ablate:
                    o_ps[(sqb, h)] = o_pool.tile(
                        [VW, SQ], f32, name=f"ops_{sqb}_{h}", tag="ops")

            def emit_all():
                units = [(sqb, h)
                         for sqb in range(nsq) for h in range(hpc)]
                for u, (sqb, h) in enumerate(units):
                    alloc_unit(sqb, h)
                    for gi, (c0, n) in enumerate(groups):
                        emit_qk_group(sqb, h, c0, n)
                        if gi in mask_at:
                            emit_mask(sqb, h, *mask_at[gi])
                        if u >= 1:
                            pq, ph = units[u - 1]
                            emit_pv(pq, ph, c0, c0 + n)
                            if gi == ng - 1:
                                emit_tail(pq, ph)
                for gi, (c0, n) in enumerate(groups):
                    pq, ph = units[-1]
                    emit_pv(pq, ph, c0, c0 + n)
                    if gi == ng - 1:
                        emit_tail(pq, ph)

            if loop_n is None:
                emit_all()
            else:
                hints = (mybir.EngineType.PE, mybir.EngineType.Activation,
                         mybir.EngineType.DVE)
                with tc.For_i(0, loop_n, 1, hint_engines=hints,
                              staggered_reset=bool(loop_stagger)):
                    emit_all()

    nc.finalize()
    return nc


def shard_inputs(K, Q, V, mask, hpc=HPC, s=S, n_cores=N_CORES,
                 act_chunks=ACT_CHUNKS):
    """Full inputs -> per-core in_maps with device-friendly host layouts."""
    nsq = s // SQ
    nck = s // CK
    n_units = n_cores * hpc
    Kf = np.asarray(K, np.float32).reshape(n_units, s, D)
    Qf = np.asarray(Q, np.float32).reshape(n_units, s, D)
    Vf = np.asarray(V, np.float32).reshape(n_units, s, D)
    keepT = (~np.asarray(mask).reshape(s, s)).T  # [k, q], True = attend
    # chunk-major [nck, CK, nsq, SQ] view of keep^T
    keep_cm = keepT.reshape(nck, CK, nsq, SQ)
    mk_vals = np.where(keep_cm, np.float32(1.0), np.float32(0.0))
    if nck == 16 and act_chunks < nck:
        # DVE-trick chunks carry the exp bias instead of 0/1
        mk_vals[act_chunks:] = np.where(
            keep_cm[act_chunks:], np.float32(TRICK_B0), np.float32(TRICK_BNEG))
    mk_host = np.ascontiguousarray(
        mk_vals.astype(np.float16)
        .transpose(2, 1, 0, 3)
        .reshape(nsq, CK, nck * SQ)
    )
    in_maps = []
    for c in range(n_cores):
        sl = slice(c * hpc, (c + 1) * hpc)
        qt = np.ascontiguousarray(Qf[sl].transpose(0, 2, 1)).astype(np.float16)
        kt = np.ascontiguousarray(Kf[sl].transpose(0, 2, 1)).astype(np.float16)
        vp = np.zeros((hpc, s, VW), np.float16)
        vp[:, :, :D] = Vf[sl]
        vp[:, :, D] = 1.0
        vp = np.ascontiguousarray(
            vp.reshape(hpc, nck, CK, VW).transpose(0, 2, 1, 3)
            .reshape(hpc, CK, nck * VW)
        )
        in_maps.append({"qt": qt, "kt": kt, "vp": vp, "mk": mk_host})
    return in_maps


_NC_CACHE = {}


def _get_nc():
    if "nc" not in _NC_CACHE:
        _NC_CACHE["nc"] = build_nc()
    return _NC_CACHE["nc"]


def run_sharded(in_maps, trace=False, **kwargs):
    return run_bass_kernel_spmd(
        _get_nc(), in_maps, core_ids=list(range(N_CORES)), trace=trace, **kwargs
    )


def unshard_output(per_core_raw, hpc=HPC, s=S):
    """[hpc, nsq, VW, SQ] raw blocks per core -> [n*hpc, s, D] normalized.

    Row D of each block is the softmax denominator Z; dividing and
    transposing here is O(S*D) host work (same order as unsharding).
    """
    n = len(per_core_raw)
    out = np.empty((n * hpc, s, D), np.float32)
    for c, o in enumerate(per_core_raw):
        ot = o[:, :, :D, :] / o[:, :, D:D + 1, :]   # [hpc, nsq, D, SQ]
        out[c * hpc:(c + 1) * hpc] = (
            ot.transpose(0, 1, 3, 2).reshape(hpc, s, D))
    return out


def assemble_output(results):
    out = unshard_output([results[c]["o"] for c in range(N_CORES)])
    return out.reshape(B, H, S, D)


def kernel(K, Q, V, mask):
    in_maps = shard_inputs(K, Q, V, mask)
    res = run_sharded(in_maps)
    return assemble_output(res.results)
